# revision 30
# baseline (speedup 1.0000x reference)
"""GCNConv Trainium2 kernel: out = segment_sum(features[src], dst) @ W + b.

Strategy (8 NeuronCores, graph partitioned by destination node):
  - Host: partition the 391 dst-node tiles (128 nodes each) across 8 cores
    (LPT balance by edge count).  Edges live with their dst tile.  Features
    are replicated to every core in bf16, split into two 25000-row tables so
    gather indices fit in int16 (dma_gather requirement).
  - Device (per core): dma_gather edge source rows (bf16, 256B rows) in
    groups of GCHUNK 128-edge chunks, spread round-robin across all 4 SWDGE
    queues with deep buffering so descriptor generation runs 4-wide on the
    GpSimd Q7 cluster; per group build all its one-hot(dst_local) matrices
    in ONE wide DVE op (broadcast-AP compare against iota); per chunk
    matmul-accumulate msgs.T @ onehot into PSUM, yielding agg.T per node
    tile; then out.T = W.T @ agg.T and a fused bias-add on Scalar; DMA
    out.T tiles to DRAM.
  - Host: transpose + scatter per-core tile outputs back to [50000, 128].
"""

import os
import sys

for _p in ("/opt/trn_rl_repo",):
    if _p not in sys.path and os.path.isdir(_p):
        sys.path.insert(0, _p)

import numpy as np
import ml_dtypes

P = 128
N_NODES = 50000
N_EDGES = 640000
D = 128
NCORES = 8
HALF = 25000          # int16 index-range split of the feature table
NTILE = (N_NODES + P - 1) // P          # 391
NSLOT = (NTILE + NCORES - 1) // NCORES  # 49 node tiles per core
GCHUNK = 25           # chunks (of 128 gathered rows) per dma_gather call
NQUEUES = 4           # SWDGE queues; gather desc-gen contexts run concurrently
GBUFS = 6             # gather group buffers per stream (deep: keep 4 desc-gens going)

BF16 = ml_dtypes.bfloat16


# ---------------------------------------------------------------- host plan

def plan(src, dst):
    """Partition node tiles across cores and lay out padded, chunked edge
    lists.  Chunk counts are shared across cores (max over cores) so the
    single SPMD program fits every core."""
    src = np.asarray(src).astype(np.int64)
    dst = np.asarray(dst).astype(np.int64)
    tile = dst // P
    cnt = np.bincount(tile, minlength=NTILE)

    # LPT assignment of tiles to cores, capacity NSLOT each
    order = np.argsort(-cnt, kind="stable")
    core_tiles = [[] for _ in range(NCORES)]
    load = np.zeros(NCORES)
    for t in order:
        for c in sorted(range(NCORES), key=lambda c: load[c]):
            if len(core_tiles[c]) < NSLOT:
                core_tiles[c].append(int(t))
                load[c] += cnt[t]
                break
    for c in range(NCORES):
        core_tiles[c].sort(key=lambda t: -cnt[t])
        while len(core_tiles[c]) < NSLOT:
            core_tiles[c].append(-1)  # dummy empty tile
        # interleave big/small so every small slot's epilogue latency hides
        # under a big neighbour's matmul stream (no run of tiny slots at the
        # end of the schedule)
        desc = core_tiles[c]
        inter = []
        i, j = 0, NSLOT - 1
        while i <= j:
            inter.append(desc[i])
            if i != j:
                inter.append(desc[j])
            i, j = i + 1, j - 1
        core_tiles[c] = inter

    # edges grouped by tile
    edge_order = np.argsort(tile, kind="stable")
    tile_sorted = tile[edge_order]
    starts = np.searchsorted(tile_sorted, np.arange(NTILE))
    ends = np.searchsorted(tile_sorted, np.arange(NTILE), side="right")

    lo_edges = [[None] * NSLOT for _ in range(NCORES)]
    hi_edges = [[None] * NSLOT for _ in range(NCORES)]
    n_lo = np.zeros((NCORES, NSLOT), np.int64)
    n_hi = np.zeros((NCORES, NSLOT), np.int64)
    for c in range(NCORES):
        for s, t in enumerate(core_tiles[c]):
            if t < 0:
                lo_edges[c][s] = hi_edges[c][s] = np.empty(0, np.int64)
                continue
            e = edge_order[starts[t]:ends[t]]
            m = src[e] < HALF
            lo_edges[c][s] = e[m]
            hi_edges[c][s] = e[~m]
            n_lo[c, s] = m.sum()
            n_hi[c, s] = (~m).sum()

    # Contiguous stream packing: slot s occupies edge positions
    # [B_s, B_s + maxn_s) of its stream (maxn shared across cores so the
    # SPMD program structure is identical); chunks of 128 run across slot
    # boundaries, and a boundary chunk is consumed by one masked matmul per
    # slot it touches.
    maxn_lo = n_lo.max(axis=0).astype(int)
    maxn_hi = n_hi.max(axis=0).astype(int)
    for s in range(NSLOT):  # every slot needs >=1 MM so PSUM is written
        if maxn_lo[s] + maxn_hi[s] == 0:
            maxn_lo[s] = 1

    Llo = _stream_layout(maxn_lo)
    Lhi = _stream_layout(maxn_hi)
    KLO, KHI = Llo["K"], Lhi["K"]
    MLO = len(Llo["mm_chunk"])

    idx = np.zeros((NCORES, KLO + KHI, P), np.int16)
    dstl = np.full((NCORES, MLO + len(Lhi["mm_chunk"]), P), -1.0, np.float32)
    for c in range(NCORES):
        for edges_c, n_arr, L, koff, moff, table_off in (
            (lo_edges[c], n_lo, Llo, 0, 0, 0),
            (hi_edges[c], n_hi, Lhi, KLO, MLO, HALF),
        ):
            K, B = L["K"], L["B"]
            si = np.zeros(K * P, np.int16)
            sd = np.full(K * P, -1.0, np.float32)
            for s, t in enumerate(core_tiles[c]):
                e = edges_c[s]
                if len(e) == 0:
                    continue
                b0 = int(B[s])
                si[b0:b0 + len(e)] = (src[e] - table_off).astype(np.int16)
                sd[b0:b0 + len(e)] = (dst[e] - t * P).astype(np.float32)
            idx[c, koff:koff + K] = si.reshape(K, P)
            for s in range(NSLOT):
                lo_pos, n_here = int(B[s]), int(n_arr[c, s])
                for m in L["mm_of_slot"][s]:
                    cm = L["mm_chunk"][m]
                    p0 = cm * P
                    a = max(p0, lo_pos)
                    b2 = min(p0 + P, lo_pos + n_here)
                    if b2 > a:
                        dstl[c, moff + m, a - p0:b2 - p0] = sd[a:b2]

    return {
        "core_tiles": core_tiles,
        "maxn_lo": tuple(int(x) for x in maxn_lo),
        "maxn_hi": tuple(int(x) for x in maxn_hi),
        "KLO": KLO, "KHI": KHI,
        "idx": idx, "dstl": dstl,
    }


def _stream_layout(maxn):
    """Contiguous per-stream layout: slot base offsets, chunk count, and the
    slot-major MM list (one MM per (slot, chunk) the slot's range touches)."""
    B = np.concatenate([[0], np.cumsum(maxn)])
    E = int(B[-1])
    K = -(-E // P)
    mm_chunk = []
    mm_of_slot = [[] for _ in range(NSLOT)]
    for s in range(NSLOT):
        if maxn[s] == 0:
            continue
        c0, c1 = int(B[s]) // P, int(B[s] + maxn[s] - 1) // P
        for c in range(c0, c1 + 1):
            mm_of_slot[s].append(len(mm_chunk))
            mm_chunk.append(c)
    return {"B": B, "E": E, "K": K, "mm_chunk": mm_chunk,
            "mm_of_slot": mm_of_slot}


def _groups(K):
    """Split stream of K chunks into gather groups: GCHUNK-sized in the main
    body, tapering to ~8-chunk groups at the end so the final wave's matmul
    backlog is short."""
    TAIL = 32  # chunks covered by small tail groups
    out = []
    c = 0
    main = max(0, K - TAIL)
    while c < main:
        out.append((c, min(c + GCHUNK, main)))
        c = out[-1][1]
    while c < K:
        out.append((c, min(c + 8, K)))
        c = out[-1][1]
    return out


def pack_gidx(idx):
    """[K,128] int16 chunk-major indices -> [128, K*8] dma_gather layout
    (index i of a group at [i%16, i//16], replicated on partitions 16..127)."""
    K = idx.shape[0]
    out = np.zeros((128, K * 8), np.int16)
    for c0, c1 in _groups(K):
        g = idx[c0:c1].reshape(-1)                # i = (c-c0)*128 + lane
        blk = g.reshape(-1, 16).T                 # [16, (c1-c0)*8]
        out[:, c0 * 8:c1 * 8] = np.tile(blk, (8, 1))
    return out


# ---------------------------------------------------------------- program

def build(maxn_lo, maxn_hi, dbg=False):
    import concourse.bass as bass
    import concourse.mybir as mybir
    from concourse import bacc
    import concourse.tile as tile
    from bisect import bisect_left

    Llo = _stream_layout(np.asarray(maxn_lo))
    Lhi = _stream_layout(np.asarray(maxn_hi))
    KLO, KHI = Llo["K"], Lhi["K"]
    MLO, MHI = len(Llo["mm_chunk"]), len(Lhi["mm_chunk"])
    NCH = KLO + KHI
    NMM = MLO + MHI
    bf16, f32, i16 = mybir.dt.bfloat16, mybir.dt.float32, mybir.dt.int16

    nc = bacc.Bacc("TRN2", debug=dbg, num_swdge_queues=NQUEUES)
    flo = nc.dram_tensor("flo", [HALF, D], bf16, kind="ExternalInput")
    fhi = nc.dram_tensor("fhi", [N_NODES - HALF, D], bf16, kind="ExternalInput")
    gidx = nc.dram_tensor("gidx", [P, NCH * 8], i16, kind="ExternalInput")
    dstl = nc.dram_tensor("dstl", [P, NMM], bf16, kind="ExternalInput")
    iota = nc.dram_tensor("iota", [P, P], bf16, kind="ExternalInput")
    wmat = nc.dram_tensor("wmat", [P, P], bf16, kind="ExternalInput")
    bcol = nc.dram_tensor("bcol", [P, 1], f32, kind="ExternalInput")
    out = nc.dram_tensor("out", [P, NSLOT * P], f32, kind="ExternalOutput")

    lo_groups, hi_groups = _groups(KLO), _groups(KHI)

    with tile.TileContext(nc) as tc:
        with tc.tile_pool(name="const", bufs=1) as cp, \
             tc.tile_pool(name="gat", bufs=GBUFS) as gp, \
             tc.tile_pool(name="oh", bufs=3) as ohp, \
             tc.tile_pool(name="res", bufs=4) as resp, \
             tc.tile_pool(name="psA", bufs=4, space="PSUM") as psA, \
             tc.tile_pool(name="psB", bufs=2, space="PSUM") as psB:

            # gidx loads: a small "head" slice per stream covering the first
            # HEAD groups unblocks the first desc-gen wave immediately; the
            # bulk follows in two big DMAs.
            HEAD = 2
            gsl = {}   # stream -> (head_tile, rest_tile, head_groups, rest_c0)
            for name, groups, coff in (("lo", lo_groups, 0),
                                       ("hi", hi_groups, KLO)):
                nhead = min(HEAD, len(groups))
                hc1 = groups[nhead - 1][1] if nhead else 0
                ht = cp.tile([P, max(hc1, 1) * 8], i16, tag="gxh" + name)
                nc.scalar.dma_start(
                    out=ht[:, :hc1 * 8],
                    in_=gidx[:, coff * 8:(coff + hc1) * 8])
                K = groups[-1][1] if groups else 0
                rt = None
                if K > hc1:
                    rt = cp.tile([P, (K - hc1) * 8], i16, tag="gxr" + name)
                gsl[name] = (ht, rt, nhead, hc1)
            iota_sb = cp.tile([P, P], bf16)
            nc.sync.dma_start(out=iota_sb[:], in_=iota[:])
            dstl_t = cp.tile([P, NMM], bf16)
            nc.sync.dma_start(out=dstl_t[:], in_=dstl[:])
            for name, coff in (("lo", 0), ("hi", KLO)):
                ht, rt, nhead, hc1 = gsl[name]
                if rt is not None:
                    K = rt.shape[1] // 8 + hc1
                    nc.sync.dma_start(
                        out=rt[:],
                        in_=gidx[:, (coff + hc1) * 8:(coff + K) * 8])
            w_t = cp.tile([P, P], bf16)
            nc.sync.dma_start(out=w_t[:], in_=wmat[:])
            b_t = cp.tile([P, 1], f32)
            nc.sync.dma_start(out=b_t[:], in_=bcol[:])

            # hoist num_idxs registers (one per distinct group size) so each
            # gather carries no register MOVE -> no WAR chain at dispatch
            sizes = sorted({c1 - c0 for c0, c1 in lo_groups + hi_groups})
            szreg = {n: nc.gpsimd.to_reg(n * P) for n in sizes}



            # per-stream gather state: (groups, table, MM/chunk metadata)
            st = {
                "lo": {"groups": lo_groups, "tab": flo, "g": 0, "L": Llo,
                       "tile": None, "oh": None, "c0": 0, "c1": 0,
                       "m0": 0, "moff": 0},
                "hi": {"groups": hi_groups, "tab": fhi, "g": 0, "L": Lhi,
                       "tile": None, "oh": None, "c0": 0, "c1": 0,
                       "m0": 0, "moff": MLO},
            }

            def fetch(S, name):
                c0, c1 = S["groups"][S["g"]]
                n = c1 - c0
                ht, rt, nhead, hc1 = gsl[name]
                koff = 0 if name == "lo" else KLO
                if S["g"] < nhead:
                    idxs = ht[:, c0 * 8:c1 * 8]
                else:
                    idxs = rt[:, (c0 - hc1) * 8:(c1 - hc1) * 8]
                t = gp.tile([P, n * P], mybir.dt.bfloat16, tag="g" + name)
                nc.gpsimd.dma_gather(
                    out_ap=t[:].rearrange("p (g d) -> p g d", d=P),
                    in_ap=S["tab"][:],
                    idxs_ap=idxs,
                    num_idxs=n * P,
                    num_idxs_reg=szreg[n],
                    elem_size=P,
                    single_packet=False,
                )
                # all one-hots for MMs whose chunk lies in [c0, c1), in ONE
                # wide DVE op: oh[p, k, j] = (dstl[p, m0+k] == j)
                mmc = S["L"]["mm_chunk"]
                m0 = bisect_left(mmc, c0)
                m1 = bisect_left(mmc, c1)
                nm = m1 - m0
                # tensor_tensor (not tensor_scalar/stt): it only runs in DVE
                # single-port mode, so it never locks GpSimd out of the SBUF
                # port pair the SWDGE descriptor rings live on
                oh = ohp.tile([P, nm * P], mybir.dt.bfloat16, tag="oh" + name)
                dc = S["moff"] + m0
                nc.vector.tensor_tensor(
                    out=oh[:].rearrange("p (c j) -> p c j", j=P),
                    in0=dstl_t[:, dc:dc + nm].unsqueeze(2).to_broadcast((P, nm, P)),
                    in1=iota_sb[:].unsqueeze(1).to_broadcast((P, nm, P)),
                    op=mybir.AluOpType.is_equal,
                )
                S["tile"], S["oh"], S["c0"], S["c1"], S["m0"] = t, oh, c0, c1, m0
                S["g"] += 1

            for s in range(NSLOT):
                nmm = len(Llo["mm_of_slot"][s]) + len(Lhi["mm_of_slot"][s])
                ps_agg = psA.tile([P, P], f32, tag="agg")
                ci = 0
                for name in ("lo", "hi"):
                    S = st[name]
                    for m in S["L"]["mm_of_slot"][s]:
                        cm = S["L"]["mm_chunk"][m]
                        while S["tile"] is None or cm >= S["c1"]:
                            fetch(S, name)
                        off = cm - S["c0"]
                        ohoff = m - S["m0"]
                        nc.tensor.matmul(
                            out=ps_agg[:],
                            lhsT=S["tile"][:, off * P:(off + 1) * P],
                            rhs=S["oh"][:, ohoff * P:(ohoff + 1) * P],
                            start=(ci == 0), stop=(ci == nmm - 1),
                        )
                        ci += 1

                aggT = resp.tile([P, P], mybir.dt.bfloat16, tag="aggT")
                nc.scalar.copy(out=aggT[:], in_=ps_agg[:])
                ps_out = psB.tile([P, P], f32, tag="out")
                nc.tensor.matmul(out=ps_out[:], lhsT=w_t[:], rhs=aggT[:],
                                 start=True, stop=True)
                o_sb = resp.tile([P, P], f32, tag="osb")
                nc.scalar.activation(
                    out=o_sb[:], in_=ps_out[:],
                    func=mybir.ActivationFunctionType.Identity,
                    bias=b_t[:, 0:1],
                )
                nc.sync.dma_start(out=out[:, s * P:(s + 1) * P], in_=o_sb[:])

    # Spread gathers across SWDGE queues.  Tile assigns each Pool-engine DMA
    # a DMASW completion lane in *scheduled* order; queue choice must be a
    # function of that lane (the sim/ucode bind each lane to one queue), so
    # retag after scheduling: queue = lane % NQUEUES.
    for inst in nc.inst_map.values():
        if isinstance(inst, mybir.InstDMAGatherAnt):
            proc = inst.bass_scheduled_proc
            if proc is not None and 11 <= proc <= 18:
                inst.queue_num = (proc - 11) % NQUEUES

    nc.compile()
    return nc


# ---------------------------------------------------------------- in_maps

def make_in_maps(features, W, b, pl):
    f16 = np.ascontiguousarray(features).astype(BF16)
    iota_np = np.tile(np.arange(P, dtype=np.float32)[None, :], (P, 1)).astype(BF16)
    w_np = np.asarray(W, np.float32).astype(BF16)
    b_np = np.asarray(b, np.float32).reshape(1, D).T.copy()  # [128,1]
    in_maps = []
    for c in range(NCORES):
        in_maps.append({
            "flo": f16[:HALF],
            "fhi": f16[HALF:],
            "gidx": pack_gidx(pl["idx"][c]),
            "dstl": np.ascontiguousarray(pl["dstl"][c].T).astype(BF16),
            "iota": iota_np,
            "wmat": w_np,
            "bcol": b_np,
        })
    return in_maps


def unshard(outs, core_tiles):
    """outs: list of {'out': [128, NSLOT*128] f32} per core -> [50000,128]."""
    full = np.zeros((N_NODES, D), np.float32)
    for c in range(NCORES):
        oT = np.asarray(outs[c]["out"], np.float32)
        for s, t in enumerate(core_tiles[c]):
            if t < 0:
                continue
            n0 = t * P
            n1 = min(n0 + P, N_NODES)
            full[n0:n1, :] = oT[:, s * P:s * P + (n1 - n0)].T
    return full


# ---------------------------------------------------------------- entry

_CACHE = {}


def kernel(features, src, dst, W, b):
    from concourse.bass_utils import run_bass_kernel_spmd

    pl = plan(src, dst)
    key = (pl["maxn_lo"], pl["maxn_hi"])
    if key not in _CACHE:
        _CACHE[key] = build(pl["maxn_lo"], pl["maxn_hi"])
    nc = _CACHE[key]
    in_maps = make_in_maps(features, W, b, pl)
    last = None
    for _ in range(3):  # retry: a previously wedged pool device can fail a load
        try:
            res = run_bass_kernel_spmd(nc, in_maps, core_ids=list(range(NCORES)))
            return unshard(res.results, pl["core_tiles"])
        except Exception as e:  # noqa: BLE001
            last = e
    raise last
